# revision 1
# baseline (speedup 1.0000x reference)
"""Multi-head attention TRN2 kernel, 8-core SPMD, v3.

Sharding: core c owns batch b=c//2 and head-group hg=c%2 (8 of 16 heads).
Each core projects Q/K/V for its 8 heads over the full 2048-token sequence
of its batch, runs attention, and computes a PARTIAL output projection
(contraction over its 512 features). The host sums the two partial outputs
per batch (the all-reduce of the tensor-parallel sharding) -- no on-device
collectives.

All matmul PSUM outputs are <= 512 f32 columns (one 2KB bank). Everything
runs in bf16 (tolerance allows it; fp8 fails: high-score keys amplify
quantization error through exp). Scores are computed in transposed [key,
query] orientation; exp on the scalar engine (scale=1/8, no max
subtraction), 0/1 mask applied multiplicatively post-exp on the vector
engine; PV accumulates [65,1024] with a ones-column denominator row;
normalization via reciprocal + partition-broadcast; odd heads reach
partitions 64..127 of the out-projection input via a small SBUF-to-SBUF
DMA shift so the output projection contracts full-128 partitions.
"""

import numpy as np

B, S, D, H, DH = 4, 2048, 1024, 16, 64
NCORES = 8

_CACHE = {}


def _build():
    from contextlib import ExitStack

    import concourse.mybir as mybir
    import concourse.tile as tile
    from concourse import bacc

    f32 = mybir.dt.float32
    bf16 = mybir.dt.bfloat16
    EXP = mybir.ActivationFunctionType.Exp
    IDENT = mybir.ActivationFunctionType.Identity

    nc = bacc.Bacc(
        "TRN2",
        target_bir_lowering=False,
        debug=False,
        enable_asserts=False,
        num_devices=NCORES,
    )

    xq_d = nc.dram_tensor("xq8", [128, 4, 2, 2, 1024], bf16, kind="ExternalInput").ap()
    xk_d = nc.dram_tensor("xk8", [128, 4, 2, 2, 1024], bf16, kind="ExternalInput").ap()
    xv_d = nc.dram_tensor("xv", [128, 16, 4, 2, 128], bf16, kind="ExternalInput").ap()
    wq_d = nc.dram_tensor("wq8", [128, 4, 4, 2, 128], bf16, kind="ExternalInput").ap()
    wk_d = nc.dram_tensor("wk8", [128, 4, 4, 2, 128], bf16, kind="ExternalInput").ap()
    wv_d = nc.dram_tensor("wv", [128, 4, 2, 2, 256], bf16, kind="ExternalInput").ap()
    wo_d = nc.dram_tensor("wo8", [128, 4, 8, 128], bf16, kind="ExternalInput").ap()
    mk_d = nc.dram_tensor("mask_t", [128, 16, S], bf16, kind="ExternalInput").ap()
    wb_d = nc.dram_tensor("wb", [128, 16], f32, kind="ExternalInput").ap()
    out_d = nc.dram_tensor("out_t", [1024, S], bf16, kind="ExternalOutput").ap()

    with tile.TileContext(nc) as tc:
        stk = ExitStack()

        kpool = stk.enter_context(tc.tile_pool(name="konst", bufs=1))
        wb_sb = kpool.tile([128, 16], f32, name="wb_sb")
        wq_sb = [
            kpool.tile([128, 4, 2, 2, 128], bf16, name=f"wq_sb{p}")
            for p in range(2)
        ]
        wk_sb = [
            kpool.tile([128, 4, 2, 2, 128], bf16, name=f"wk_sb{p}")
            for p in range(2)
        ]
        wv_sb = kpool.tile([128, 4, 2, 2, 256], bf16, name="wv_sb")
        wo_sb = kpool.tile([128, 4, 8, 128], bf16, name="wo_sb")

        mpool = stk.enter_context(tc.tile_pool(name="msk", bufs=1))
        qkpool = stk.enter_context(tc.tile_pool(name="qk8", bufs=1))
        qp = [qkpool.tile([128, 2, 1024], bf16, name=f"qp_{fb}") for fb in range(4)]
        kp = [qkpool.tile([128, 16, 128], bf16, name=f"kp_{fb}") for fb in range(4)]
        vpool = stk.enter_context(tc.tile_pool(name="vsb", bufs=1))
        v_sb = [vpool.tile([128, 8, 65], bf16, name=f"v_{kc}") for kc in range(16)]
        xapool = stk.enter_context(tc.tile_pool(name="xatt", bufs=1))
        x_att = [
            xapool.tile([128, 4, 1024], bf16, name=f"x_att_{th}")
            for th in range(2)
        ]
        hstage = xapool.tile([64, 1024], bf16, name="hstage")

        pepool = stk.enter_context(tc.tile_pool(name="pep", bufs=17))
        rpool = stk.enter_context(tc.tile_pool(name="rcp", bufs=2))
        bpool = stk.enter_context(tc.tile_pool(name="rbp", bufs=1))
        opool = stk.enter_context(tc.tile_pool(name="ost", bufs=1))
        xvpool = stk.enter_context(tc.tile_pool(name="xvs", bufs=2))

        xistk = ExitStack()
        xipool = xistk.enter_context(tc.tile_pool(name="xi8", bufs=3))

        stpool = stk.enter_context(tc.tile_pool(name="pst", bufs=2, space="PSUM"))
        xtpool = stk.enter_context(tc.tile_pool(name="pxt", bufs=2, space="PSUM"))

        # ---- constant DMAs --------------------------------------------
        nc.sync.dma_start(wb_sb[:], wb_d[:, :])
        nc.sync.dma_start(wq_sb[0][:], wq_d[:, :, 0:2, :, :])
        nc.scalar.dma_start(wk_sb[0][:], wk_d[:, :, 0:2, :, :])
        mk_t = [None] * 16

        for kc in range(16):
            nc.vector.memset(v_sb[kc][:, :, 64:65], 1.0)

        # ---- Q/K projections: fb pairs, 4 concurrent chains -----------
        def proj_pair(fbs, xsrc, wsb, dst, bc0, deng, qstyle):
            pss = {}
            for idx, fb in enumerate(fbs):
                for th in range(2):
                    pool = stpool if idx == 0 else xtpool
                    tag = "st" if idx == 0 else "xt"
                    pss[(fb, th)] = pool.tile(
                        [128, 1024], f32, tag=tag, name="pp"
                    )
            for ic in range(4):
                xt_i = xipool.tile(
                    [128, 2, 2, 1024], bf16, tag="xi", name="xi"
                )
                deng.dma_start(xt_i[:], xsrc[:, ic, :, :, :])
                for fb in fbs:
                    for th in range(2):
                        for nh in range(2):
                            nsl = slice(nh * 512, (nh + 1) * 512)
                            for i in range(2):
                                nc.tensor.matmul(
                                    pss[(fb, th)][:, nsl],
                                    lhsT=wsb[:, ic, fb - fbs[0], i, :],
                                    rhs=xt_i[:, th, i, nsl],
                                    start=(ic == 0 and i == 0),
                                    stop=(ic == 3 and i == 1),
                                )
            for fb in fbs:
                for th in range(2):
                    if qstyle:
                        nc.vector.tensor_scalar_add(
                            dst[fb][:, th, :],
                            pss[(fb, th)][:],
                            wb_sb[:, bc0 + fb : bc0 + fb + 1],
                        )
                    else:
                        nc.vector.tensor_scalar_add(
                            dst[fb][:, th * 8 : (th + 1) * 8, :],
                            pss[(fb, th)][:].rearrange(
                                "p (a b) -> p a b", b=128
                            ),
                            wb_sb[:, bc0 + fb : bc0 + fb + 1],
                        )

        xv_cache = {}

        def vchain(kc):
            kc2 = kc // 2
            if kc2 not in xv_cache:
                xv_t = xvpool.tile(
                    [128, 2, 4, 2, 128], bf16, tag="xv", name="xv_t"
                )
                nc.gpsimd.dma_start(
                    xv_t[:], xv_d[:, 2 * kc2 : 2 * kc2 + 2, :, :, :]
                )
                xv_cache[kc2] = xv_t
            xv_t = xv_cache[kc2]
            ps = stpool.tile([128, 1024], f32, tag="st", name="pv")
            for ici in range(8):
                ic, i = ici // 2, ici % 2
                nc.tensor.matmul(
                    ps[:, 0:512],
                    lhsT=xv_t[:, kc % 2, ic, i, :],
                    rhs=wv_sb[:, ic, :, i, :],
                    start=(ici == 0),
                    stop=(ici == 7),
                )
            nc.vector.tensor_copy(
                v_sb[kc][:, :, 0:64],
                ps[:, 0:512].rearrange("p (h f) -> p h f", f=64),
            )

        def unit(h, qh):
            fb, j = h // 2, h % 2
            pe_tiles = []
            for kc in range(16):
                st = stpool.tile([128, 1024], f32, tag="st", name="st")
                for nh in range(2):
                    nsl = slice(nh * 512, (nh + 1) * 512)
                    nc.tensor.matmul(
                        st[:, nsl],
                        lhsT=kp[fb][64 * j : 64 * j + 64, kc, :],
                        rhs=qp[fb][64 * j : 64 * j + 64, qh, nsl],
                        start=True,
                        stop=True,
                    )
                pe = pepool.tile([128, 1024], bf16, tag="pe", name="pe")
                nc.scalar.activation(pe[:], st[:], EXP, scale=0.125)
                nc.vector.tensor_mul(pe[:], pe[:], mk_t[kc][:])
                pe_tiles.append(pe)
            xt = xtpool.tile([128, 1024], f32, tag="xt", name="xt")
            for nh in range(2):
                nsl = slice(nh * 512, (nh + 1) * 512)
                for kc in range(16):
                    nc.tensor.matmul(
                        xt[0:65, nsl],
                        lhsT=v_sb[kc][:, h, :],
                        rhs=pe_tiles[kc][:, nsl],
                        start=(kc == 0),
                        stop=(kc == 15),
                    )
            rb = bpool.tile([64, 1024], f32, tag="rb", name="rb")
            for rh in range(2):
                rsl = slice(rh * 512, (rh + 1) * 512)
                rc = rpool.tile([1, 512], f32, tag="rc", name="rc")
                nc.vector.reciprocal(rc[:], xt[64:65, rsl])
                nc.gpsimd.partition_broadcast(rb[:, rsl], rc[:])
            if j == 0:
                nc.vector.tensor_mul(
                    x_att[qh][0:64, fb, :], xt[0:64, :], rb[:]
                )
            else:
                nc.vector.tensor_mul(hstage[:], xt[0:64, :], rb[:])
                nc.sync.dma_start(x_att[qh][64:128, fb, :], hstage[:])

        def oproj(ofb, th):
            ps = xtpool.tile([128, 1024], f32, tag="xt", name="po")
            for nh in range(2):
                nsl = slice(nh * 512, (nh + 1) * 512)
                for ic in range(4):
                    nc.tensor.matmul(
                        ps[:, nsl],
                        lhsT=wo_sb[:, ic, ofb, :],
                        rhs=x_att[th][:, ic, nsl],
                        start=(ic == 0),
                        stop=(ic == 3),
                    )
            for nh in range(2):
                nsl = slice(nh * 512, (nh + 1) * 512)
                co = opool.tile([128, 512], bf16, tag=f"co{nh}", name="co")
                if th == 1 and ofb % 2 == 0:
                    nc.scalar.activation(
                        co[:], ps[:, nsl], IDENT,
                        bias=wb_sb[:, 8 + ofb : 9 + ofb],
                    )
                else:
                    nc.vector.tensor_scalar_add(
                        co[:], ps[:, nsl], wb_sb[:, 8 + ofb : 9 + ofb]
                    )
                deng = nc.sync if (ofb + nh) % 2 == 0 else nc.scalar
                deng.dma_start(
                    out_d[
                        ofb * 128 : (ofb + 1) * 128,
                        th * 1024 + nh * 512 : th * 1024 + (nh + 1) * 512,
                    ],
                    co[:],
                )

        # ---- emission --------------------------------------------------
        proj_pair((0, 1), xq_d, wq_sb[0], qp, 0, nc.sync, True)
        proj_pair((0, 1), xk_d, wk_sb[0], kp, 4, nc.scalar, False)
        nc.sync.dma_start(wq_sb[1][:], wq_d[:, :, 2:4, :, :])
        nc.scalar.dma_start(wk_sb[1][:], wk_d[:, :, 2:4, :, :])
        nc.gpsimd.dma_start(wv_sb[:], wv_d[:, :, :, :, :])
        for kc in range(16):
            mt = mpool.tile([128, 1024], bf16, tag=f"mk{kc}", name=f"mk{kc}")
            deng = nc.scalar if kc < 8 else nc.sync
            deng.dma_start(mt[:], mk_d[:, kc, 0:1024])
            mk_t[kc] = mt
        nc.sync.dma_start(wo_sb[:], wo_d[:, :, :, :])
        for kc in range(16):
            vchain(kc)
        proj_pair((2, 3), xq_d, wq_sb[1], qp, 0, nc.sync, True)
        proj_pair((2, 3), xk_d, wk_sb[1], kp, 4, nc.scalar, False)
        xistk.close()

        for h in range(8):
            unit(h, 0)

        for kc in range(16):
            mt = mpool.tile([128, 1024], bf16, tag=f"mk{kc}", name=f"mk{kc}b")
            nc.sync.dma_start(mt[:], mk_d[:, kc, 1024:2048])
            mk_t[kc] = mt

        for h in range(8):
            unit(h, 1)
            oproj(h, 0)
        for ofb in range(8):
            oproj(ofb, 1)
        stk.close()

    nc.compile()
    return nc


def _get_nc():
    if "nc" not in _CACHE:
        _CACHE["nc"] = _build()
    return _CACHE["nc"]


def _prep(query, key, value, mask, Wq, bq, Wk, bk, Wv, bv, Wo, bo):
    import ml_dtypes

    f = np.float32
    b16 = ml_dtypes.bfloat16

    def x16(x2d):  # [2048 t, 1024 d] -> [128 p, 4 ic, 2 th, 2 i, 1024 t]
        xt = np.ascontiguousarray(np.asarray(x2d, f).T)  # [1024 d, 2048]
        a = xt.reshape(4, 2, 128, 2, 1024)  # ic, i, p, th, t
        return np.ascontiguousarray(a.transpose(2, 0, 3, 1, 4)).astype(b16)

    def w16(Ws):  # [512 f, 1024 d] -> [128 p, 4 ic, 4 fb, 2 i, 128 f]
        wt = np.ascontiguousarray(np.asarray(Ws, f).T)  # [1024 d, 512 f]
        return np.ascontiguousarray(
            wt.reshape(4, 2, 128, 4, 128).transpose(2, 0, 3, 1, 4)
        ).astype(b16)

    m2 = np.asarray(mask)[0, 0]  # [Sq, Sk]
    mask_t = np.ascontiguousarray(
        np.ascontiguousarray(m2.T).reshape(16, 128, S).transpose(1, 0, 2)
    ).astype(b16)

    Wq, Wk, Wv, Wo = (np.asarray(a, f) for a in (Wq, Wk, Wv, Wo))
    bq, bk, bv, bo = (np.asarray(a, f) for a in (bq, bk, bv, bo))
    bo_eff = (
        np.asarray(bo, np.float64)
        + np.asarray(Wo, np.float64) @ np.asarray(bv, np.float64)
    ).astype(f)

    in_maps = []
    for c in range(NCORES):
        b, hg = c // 2, c % 2
        hs = hg * 512
        wvs = Wv[hs : hs + 512, :]  # [512 f, 1024 d]
        wv_t = np.ascontiguousarray(
            wvs.T.reshape(4, 2, 128, 2, 256).transpose(2, 0, 3, 1, 4)
        ).astype(b16)
        wos = np.ascontiguousarray(Wo.T[hs : hs + 512, :])  # [512 d, 1024 o]
        wo16 = np.ascontiguousarray(
            wos.reshape(4, 128, 8, 128).transpose(1, 0, 2, 3)
        ).astype(b16)
        vt = np.ascontiguousarray(np.asarray(value)[b].astype(f).T)
        xv_t = np.ascontiguousarray(
            vt.reshape(4, 2, 128, 16, 128).transpose(2, 3, 0, 1, 4)
        ).astype(b16)
        wb = np.zeros((128, 16), f)
        wb[:, 0:4] = bq[hs : hs + 512].reshape(4, 128).T
        wb[:, 4:8] = bk[hs : hs + 512].reshape(4, 128).T
        if hg == 0:
            wb[:, 8:16] = bo_eff.reshape(8, 128).T
        in_maps.append(
            {
                "xq8": x16(np.asarray(query)[b]),
                "xk8": x16(np.asarray(key)[b]),
                "xv": xv_t,
                "wq8": w16(Wq[hs : hs + 512, :]),
                "wk8": w16(Wk[hs : hs + 512, :]),
                "wv": wv_t,
                "wo8": wo16,
                "mask_t": mask_t,
                "wb": np.ascontiguousarray(wb),
            }
        )
    return in_maps


def kernel(**inputs):
    from concourse.bass_utils import run_bass_kernel_spmd

    np_inputs = {k: np.asarray(v) for k, v in inputs.items()}
    in_maps = _prep(**np_inputs)
    nc = _get_nc()
    res = run_bass_kernel_spmd(nc, in_maps, list(range(NCORES)))
    out = np.empty((B, S, D), np.float32)
    for b in range(B):
        p0 = res.results[2 * b]["out_t"].astype(np.float32)
        p1 = res.results[2 * b + 1]["out_t"].astype(np.float32)
        out[b] = (p0 + p1).T
    return out



# revision 9
# speedup vs baseline: 1.1760x; 1.1760x over previous
"""Multi-head attention TRN2 kernel, 8-core SPMD, v4.

Sharding: core c owns batch b=c//2 and head-group hg=c%2 (8 of 16 heads).
Each core projects Q/K/V for its 8 heads over the full 2048-token sequence
of its batch, runs attention, and computes a PARTIAL output projection
(contraction over its 512 features). The host sums the two partial outputs
per batch -- no on-device collectives.

v4 redesign vs v3 (469 us -> target ~300 us):
- PV is computed in [queries, dims] orientation (probs tile as the
  stationary lhsT, V as moving rhs with a ones column for the softmax
  denominator): full 128 output partitions instead of 65, halving the
  tensor-engine cost of PV.
- Attention output is transposed back to [dims, queries] for the output
  projection with PE transposes; odd heads land on partitions 64..127
  directly via tile_position=(0, 64) (no SBUF-shift DMA).
- Normalization is a per-partition reciprocal + tensor_scalar multiply on
  the vector engine (no gpsimd partition_broadcast).
- Flash-style software pipeline: a short prefix (K/Q for head pair 0 and
  half the V chains), then per-(head, query-half) units where the scalar
  engine's exp (the pacing engine, ~1.04us per [128,1024] tile) is kept
  fed while the tensor engine interleaves scores, lagged PV, and
  background projection/output chains drained between score tiles.
- Output partials are written f32 (DMA has slack; removes bf16 rounding).
"""

import numpy as np
from collections import deque

B, S, D, H, DH = 4, 2048, 1024, 16, 64
NCORES = 8

V_PREFIX = 8   # V chains computed in the prefix (of 16)
LAG = 3        # PV lags scores by this many kc steps

_CACHE = {}


def _build():
    from contextlib import ExitStack

    import concourse.mybir as mybir
    import concourse.tile as tile
    from concourse import bacc

    f32 = mybir.dt.float32
    bf16 = mybir.dt.bfloat16
    EXP = mybir.ActivationFunctionType.Exp

    nc = bacc.Bacc(
        "TRN2",
        target_bir_lowering=False,
        debug=False,
        enable_asserts=False,
        num_devices=NCORES,
    )

    xq_d = nc.dram_tensor("xq8", [128, 4, 2, 2, 1024], bf16, kind="ExternalInput").ap()
    xk_d = nc.dram_tensor("xk8", [128, 4, 2, 2, 1024], bf16, kind="ExternalInput").ap()
    xv_d = nc.dram_tensor("xv", [128, 16, 4, 2, 128], bf16, kind="ExternalInput").ap()
    wq_d = nc.dram_tensor("wq8", [128, 4, 4, 2, 128], bf16, kind="ExternalInput").ap()
    wk_d = nc.dram_tensor("wk8", [128, 4, 4, 2, 128], bf16, kind="ExternalInput").ap()
    wv_d = nc.dram_tensor("wv", [128, 4, 2, 2, 256], bf16, kind="ExternalInput").ap()
    wo_d = nc.dram_tensor("wo8", [128, 4, 8, 128], bf16, kind="ExternalInput").ap()
    mk_d = nc.dram_tensor("mask_t", [128, 16, S], bf16, kind="ExternalInput").ap()
    wb_d = nc.dram_tensor("wb", [128, 16], f32, kind="ExternalInput").ap()
    id_d = nc.dram_tensor("ident", [128, 128], bf16, kind="ExternalInput").ap()
    out_d = nc.dram_tensor("out_t", [1024, S], f32, kind="ExternalOutput").ap()

    with tile.TileContext(nc) as tc:
        stk = ExitStack()

        konst = stk.enter_context(tc.tile_pool(name="konst", bufs=1))
        wb_sb = konst.tile([128, 16], f32, name="wb_sb")
        wq_sb = konst.tile([128, 4, 4, 2, 128], bf16, name="wq_sb")
        wk_sb = konst.tile([128, 4, 4, 2, 128], bf16, name="wk_sb")
        wv_sb = konst.tile([128, 4, 2, 2, 256], bf16, name="wv_sb")
        wo_sb = konst.tile([128, 4, 8, 128], bf16, name="wo_sb")
        id_sb = konst.tile([128, 128], bf16, name="id_sb")
        zz_sb = konst.tile([1, 512], bf16, name="zz_sb")

        mpool = stk.enter_context(tc.tile_pool(name="msk", bufs=1))
        mk = [mpool.tile([128, 1024], bf16, name=f"mk{kc}") for kc in range(16)]

        qkpool = stk.enter_context(tc.tile_pool(name="qk", bufs=1))
        qp = [qkpool.tile([128, 2, 1024], bf16, name=f"qp{fb}") for fb in range(4)]
        kp = [qkpool.tile([128, 16, 128], bf16, name=f"kp{fb}") for fb in range(4)]

        vpool = stk.enter_context(tc.tile_pool(name="vsb", bufs=1))
        v_sb = [vpool.tile([128, 8, 65], bf16, name=f"v{kc}") for kc in range(16)]

        xapool = stk.enter_context(tc.tile_pool(name="xatt", bufs=1))
        x_att = [xapool.tile([128, 4, 1024], bf16, name=f"xa{qh}") for qh in range(2)]

        xvpool = stk.enter_context(tc.tile_pool(name="xvs", bufs=2))
        pepool = stk.enter_context(tc.tile_pool(name="pe", bufs=10))
        xnpool = stk.enter_context(tc.tile_pool(name="xn", bufs=2))
        copool = stk.enter_context(tc.tile_pool(name="co", bufs=3))

        xbpool = stk.enter_context(tc.tile_pool(name="xb", bufs=3))

        stpool = stk.enter_context(tc.tile_pool(name="pst", bufs=2, space="PSUM"))
        pvpool = stk.enter_context(tc.tile_pool(name="ppv", bufs=2, space="PSUM"))
        trpool = stk.enter_context(tc.tile_pool(name="ptr", bufs=1, space="PSUM"))
        bgpool = stk.enter_context(tc.tile_pool(name="pbg", bufs=1, space="PSUM"))

        sp, gq, dv, pe_e, ac = nc.sync, nc.gpsimd, nc.vector, nc.tensor, nc.scalar

        # ---- constant / input DMAs (prefix-critical first) ------------
        # sp and ac are HW-DGE queues (fast dispatch); gq is SWDGE (~1us
        # per dispatch) and only carries non-critical bulk.
        sp.dma_start(wk_sb[:, :, 0:2, :, :], wk_d[:, :, 0:2, :, :])
        ac.dma_start(wq_sb[:, :, 0:2, :, :], wq_d[:, :, 0:2, :, :])
        ac.dma_start(wv_sb[:], wv_d[:, :, :, :, :])
        ac.dma_start(wb_sb[:], wb_d[:, :])
        ac.dma_start(id_sb[:], id_d[:, :])
        for kc in range(16):
            deng = ac if kc < 8 else gq
            deng.dma_start(mk[kc][:], mk_d[:, kc, 0:1024])
        gq.dma_start(wk_sb[:, :, 2:4, :, :], wk_d[:, :, 2:4, :, :])
        gq.dma_start(wq_sb[:, :, 2:4, :, :], wq_d[:, :, 2:4, :, :])
        gq.dma_start(wo_sb[:], wo_d[:, :, :, :])
        for kc in range(16):
            gq.memset(v_sb[kc][:, :, 64:65], 1.0)
        gq.memset(zz_sb[:], 0.0)

        # ---- helpers --------------------------------------------------
        _xq_cnt = [0]

        def xchunk(src_d, tb, deng):
            """Load x^T block [1024 d, 512 t] for tokens [tb*512, +512)."""
            t = xbpool.tile([128, 4, 2, 512], bf16, tag="xb", name="xb")
            deng.dma_start(
                t[:],
                src_d[:, :, tb // 2, :, (tb % 2) * 512:(tb % 2) * 512 + 512])
            return t

        def k_wide(fb, half, ca, cb):
            """K proj for keys [half*1024, +1024) of head-pair fb (stpool)."""
            ps = stpool.tile([128, 1024], f32, tag="st", name="kw")
            for g, ch in enumerate((ca, cb)):
                for ici in range(8):
                    ic, i = ici // 2, ici % 2
                    pe_e.matmul(
                        ps[:, g * 512:(g + 1) * 512],
                        lhsT=wk_sb[:, ic, fb, i, :],
                        rhs=ch[:, ic, i, :],
                        start=(ici == 0), stop=(ici == 7),
                    )
            dv.tensor_scalar_add(
                kp[fb][:, half * 8:(half + 1) * 8, :],
                ps[:].rearrange("p (a b) -> p a b", b=128),
                wb_sb[:, 4 + fb:5 + fb],
            )

        def q_wide(fb, th, ca, cb):
            """Q proj for all 1024 tokens of half th, head-pair fb (stpool)."""
            ps = stpool.tile([128, 1024], f32, tag="st", name="qw")
            for g, ch in enumerate((ca, cb)):
                for ici in range(8):
                    ic, i = ici // 2, ici % 2
                    pe_e.matmul(
                        ps[:, g * 512:(g + 1) * 512],
                        lhsT=wq_sb[:, ic, fb, i, :],
                        rhs=ch[:, ic, i, :],
                        start=(ici == 0), stop=(ici == 7),
                    )
            dv.tensor_scalar_add(
                qp[fb][:, th, :], ps[:], wb_sb[:, fb:fb + 1])

        xv_t = [None] * 16

        def v_dma(kc):
            xv_t[kc] = xvpool.tile([128, 4, 2, 128], bf16, tag="xv", name="xvt")
            gq.dma_start(xv_t[kc][:], xv_d[:, kc, :, :, :])

        def v_wide(kc0):
            """V proj for key chunks kc0, kc0+1 (stpool)."""
            ps = stpool.tile([128, 1024], f32, tag="st", name="vw")
            for g in range(2):
                kc = kc0 + g
                for ici in range(8):
                    ic, i = ici // 2, ici % 2
                    pe_e.matmul(
                        ps[:, g * 512:(g + 1) * 512],
                        lhsT=xv_t[kc][:, ic, i, :],
                        rhs=wv_sb[:, ic, :, i, :],
                        start=(ici == 0), stop=(ici == 7),
                    )
            for g in range(2):
                dv.tensor_copy(
                    v_sb[kc0 + g][:, :, 0:64],
                    ps[:, g * 512:(g + 1) * 512].rearrange(
                        "p (h f) -> p h f", f=64),
                )

        # ---- background task generators (bgpool [128,512] chains) -----
        _alt = [0]

        def _deng():
            _alt[0] ^= 1
            return sp if _alt[0] else gq

        def k_one(fb, kb):
            ch = xchunk(xk_d, kb, _deng())
            yield
            yield
            ps = bgpool.tile([128, 512], f32, tag="bg", name="kc_ps")
            for step in range(4):
                for s in range(2):
                    ici = step * 2 + s
                    ic, i = ici // 2, ici % 2
                    pe_e.matmul(
                        ps[:],
                        lhsT=wk_sb[:, ic, fb, i, :],
                        rhs=ch[:, ic, i, :],
                        start=(ici == 0), stop=(ici == 7),
                    )
                yield
            dv.tensor_scalar_add(
                kp[fb][:, kb * 4:(kb + 1) * 4, :],
                ps[:].rearrange("p (a b) -> p a b", b=128),
                wb_sb[:, 4 + fb:5 + fb],
            )

        def q_one(fb, tb, ch=None):
            if ch is None:
                ch = xchunk(xq_d, tb, _deng())
                yield
                yield
            ps = bgpool.tile([128, 512], f32, tag="bg", name="qc_ps")
            for step in range(4):
                for s in range(2):
                    ici = step * 2 + s
                    ic, i = ici // 2, ici % 2
                    pe_e.matmul(
                        ps[:],
                        lhsT=wq_sb[:, ic, fb, i, :],
                        rhs=ch[:, ic, i, :],
                        start=(ici == 0), stop=(ici == 7),
                    )
                yield
            th, hh = tb // 2, tb % 2
            dv.tensor_scalar_add(
                qp[fb][:, th, hh * 512:(hh + 1) * 512], ps[:],
                wb_sb[:, fb:fb + 1])

        def q_tb_group(tb):
            """Q proj of token block tb for ALL head pairs (one chunk DMA)."""
            ch = xchunk(xq_d, tb, _deng())
            yield
            yield
            for fb in range(4):
                yield from q_one(fb, tb, ch)

        def v_chain(kc):
            v_dma(kc)
            yield
            ps = bgpool.tile([128, 512], f32, tag="bg", name="vc_ps")
            for step in range(4):
                for s in range(2):
                    ici = step * 2 + s
                    ic, i = ici // 2, ici % 2
                    pe_e.matmul(
                        ps[:],
                        lhsT=xv_t[kc][:, ic, i, :],
                        rhs=wv_sb[:, ic, :, i, :],
                        start=(ici == 0), stop=(ici == 7),
                    )
                yield
            dv.tensor_copy(
                v_sb[kc][:, :, 0:64],
                ps[:].rearrange("p (h f) -> p h f", f=64))

        def o_chain(qh, ofb, nh):
            ps = bgpool.tile([128, 512], f32, tag="bg", name="oc_ps")
            nsl = slice(nh * 512, (nh + 1) * 512)
            for ic in range(4):
                pe_e.matmul(
                    ps[:], lhsT=wo_sb[:, ic, ofb, :],
                    rhs=x_att[qh][:, ic, nsl],
                    start=(ic == 0), stop=(ic == 3),
                )
                if ic % 2 == 1:
                    yield
            co = copool.tile([128, 512], f32, tag="co", name="co")
            dv.tensor_scalar_add(co[:], ps[:], wb_sb[:, 8 + ofb:9 + ofb])
            sp.dma_start(
                out_d[ofb * 128:(ofb + 1) * 128,
                      qh * 1024 + nh * 512: qh * 1024 + (nh + 1) * 512],
                co[:],
            )

        # ---- prefix ---------------------------------------------------
        ck01 = [xchunk(xk_d, 0, sp), xchunk(xk_d, 1, sp)]
        ck23 = [xchunk(xk_d, 2, ac), xchunk(xk_d, 3, ac)]
        cq01 = [xchunk(xq_d, 0, sp), xchunk(xq_d, 1, ac)]
        k_wide(0, 0, ck01[0], ck01[1])
        k_wide(0, 1, ck23[0], ck23[1])
        q_wide(0, 0, cq01[0], cq01[1])
        for kc in range(0, V_PREFIX, 2):
            v_dma(kc)
            v_dma(kc + 1)
            v_wide(kc)

        vgens = {kc: v_chain(kc) for kc in range(V_PREFIX, 16)}
        fbgens = {}
        for fb in range(1, 4):
            fbgens[fb] = [k_one(fb, kb) for kb in range(4)]
            fbgens[fb] += [q_one(fb, tb) for tb in range(2)]
        qth1gens = [q_tb_group(tb) for tb in (2, 3)]
        proj_gens = (list(vgens.values())
                     + [g for fb in (1, 2, 3) for g in fbgens[fb]]
                     + qth1gens)
        bg = deque(proj_gens)

        def drain(n):
            done = 0
            while done < n and bg:
                try:
                    next(bg[0])
                    done += 1
                except StopIteration:
                    bg.popleft()

        def drain_gen(gen):
            """Force-finish one generator immediately."""
            for _ in gen:
                pass

        # ---- units ----------------------------------------------------
        # Cross-unit pipeline: unit u's PV tail, normalization, transposes
        # and x_att copy are emitted during unit u+1's first score steps so
        # the scalar engine is never starved at unit boundaries.
        pv_state = {}
        pe_q = deque()
        tr_cur = [None]

        def pv_step(u_, kc, pe):
            if kc in vgens:
                drain_gen(vgens.pop(kc))
            h_ = u_ % 8
            pv = pv_state[u_]
            for g in range(2):
                for qq in range(4):
                    pe_e.matmul(
                        pv[g][:, qq, :],
                        lhsT=pe[:, (4 * g + qq) * 128:(4 * g + qq + 1) * 128],
                        rhs=v_sb[kc][:, h_, :],
                        start=False, stop=False,
                        skip_group_check=True,
                    )

        def pv_flush(n=1):
            for _ in range(n):
                if pe_q:
                    pv_step(*pe_q.popleft())

        def finish_unit(u_):
            qh, h = u_ // 8, u_ % 8
            fb, j = h // 2, h % 2
            pv = pv_state.pop(u_)
            if j == 0:
                tr_cur[0] = trpool.tile([128, 1024], bf16, tag="tr", name="tr")
            tr = tr_cur[0]
            for g in range(2):
                xn = xnpool.tile([128, 4, 64], bf16, tag=f"xn{g}", name="xn")
                rc = xnpool.tile([128, 4, 1], f32, tag=f"rc{g}", name="rc")
                dv.reciprocal(rc[:], pv[g][:, :, 64:65])
                for qq in range(4):
                    dv.tensor_scalar_mul(
                        xn[:, qq, :], pv[g][:, qq, 0:64], rc[:, qq, :])
                for qq in range(4):
                    c = 4 * g + qq
                    pe_e.transpose(
                        tr[64 * j:64 * j + 64, c * 128:(c + 1) * 128],
                        xn[:, qq, :], id_sb[:],
                        tile_position=(0, 64 * j),
                    )
            if j == 1:
                dv.tensor_copy(x_att[qh][:, fb, :], tr[:])

        for u in range(16):
            qh, h = u // 8, u % 8
            fb, j = h // 2, h % 2
            if fb in fbgens:  # kp/qp writes must be emitted before reads
                for g_ in fbgens.pop(fb):
                    drain_gen(g_)
            if qh == 1 and qth1gens:
                for g_ in qth1gens:
                    drain_gen(g_)
                qth1gens = []
            pv_state[u] = [
                pvpool.tile([128, 4, 65], f32, tag="pv", name=f"pv{g}")
                for g in range(2)]
            for g in range(2):
                # claim the PSUM bank: start=True marks the whole 2KB zero
                # region pending and this instruction writes zeros over the
                # accumulator bytes; the PV matmuls then accumulate with
                # start=False (one hardware group per bank).
                pe_e.matmul(
                    pv_state[u][g][:],
                    lhsT=zz_sb[0:1, 0:128], rhs=zz_sb[0:1, 0:260],
                    start=True, stop=True,
                )
            for kc in range(16):
                st = stpool.tile([128, 1024], f32, tag="st", name="st")
                for nhs in range(2):
                    nsl = slice(nhs * 512, (nhs + 1) * 512)
                    pe_e.matmul(
                        st[:, nsl],
                        lhsT=kp[fb][64 * j:64 * j + 64, kc, :],
                        rhs=qp[fb][64 * j:64 * j + 64, qh, nsl],
                        start=True, stop=True,
                    )
                pe = pepool.tile([128, 1024], bf16, tag="pe", name="pe")
                ac.activation(pe[:], st[:], EXP, scale=0.125)
                dv.tensor_mul(pe[:], pe[:], mk[kc][:])
                if qh == 0 and h == 7:
                    sp.dma_start(mk[kc][:], mk_d[:, kc, 1024:2048])
                pe_q.append((u, kc, pe))
                drain(1)
                if len(pe_q) > LAG:
                    pv_flush(1)
                if kc == LAG and u > 0:
                    finish_unit(u - 1)
                if u == 8 and kc == LAG + 2:
                    for ofb in range(8):
                        for nh in range(2):
                            bg.append(o_chain(0, ofb, nh))
        while pe_q:
            pv_flush(1)
            drain(1)
        finish_unit(15)
        while bg:
            drain_gen(bg.popleft())

        # ---- tail: qh1 output projection via wide stpool tiles --------
        for ofb in range(8):
            ps = stpool.tile([128, 1024], f32, tag="st", name="ow")
            for nh in range(2):
                for ic in range(4):
                    pe_e.matmul(
                        ps[:, nh * 512:(nh + 1) * 512],
                        lhsT=wo_sb[:, ic, ofb, :],
                        rhs=x_att[1][:, ic, nh * 512:(nh + 1) * 512],
                        start=(ic == 0), stop=(ic == 3),
                    )
            for nh in range(2):
                co = copool.tile([128, 512], f32, tag="co", name="co")
                dv.tensor_scalar_add(
                    co[:], ps[:, nh * 512:(nh + 1) * 512],
                    wb_sb[:, 8 + ofb:9 + ofb])
                sp.dma_start(
                    out_d[ofb * 128:(ofb + 1) * 128,
                          1024 + nh * 512:1024 + (nh + 1) * 512],
                    co[:],
                )
        stk.close()

    nc.compile()
    return nc


def _get_nc():
    if "nc" not in _CACHE:
        _CACHE["nc"] = _build()
    return _CACHE["nc"]


def _prep(query, key, value, mask, Wq, bq, Wk, bk, Wv, bv, Wo, bo):
    import ml_dtypes

    f = np.float32
    b16 = ml_dtypes.bfloat16

    def x16(x2d):  # [2048 t, 1024 d] -> [128 p, 4 ic, 2 th, 2 i, 1024 t]
        xt = np.ascontiguousarray(np.asarray(x2d, f).T)  # [1024 d, 2048]
        a = xt.reshape(4, 2, 128, 2, 1024)  # ic, i, p, th, t
        return np.ascontiguousarray(a.transpose(2, 0, 3, 1, 4)).astype(b16)

    def w16(Ws):  # [512 f, 1024 d] -> [128 p, 4 ic, 4 fb, 2 i, 128 f]
        wt = np.ascontiguousarray(np.asarray(Ws, f).T)  # [1024 d, 512 f]
        return np.ascontiguousarray(
            wt.reshape(4, 2, 128, 4, 128).transpose(2, 0, 3, 1, 4)
        ).astype(b16)

    m2 = np.asarray(mask)[0, 0]  # [Sq, Sk]
    mask_t = np.ascontiguousarray(
        np.ascontiguousarray(m2.T).reshape(16, 128, S).transpose(1, 0, 2)
    ).astype(b16)
    ident = np.eye(128).astype(b16)

    Wq, Wk, Wv, Wo = (np.asarray(a, f) for a in (Wq, Wk, Wv, Wo))
    bq, bk, bv, bo = (np.asarray(a, f) for a in (bq, bk, bv, bo))
    bo_eff = (
        np.asarray(bo, np.float64)
        + np.asarray(Wo, np.float64) @ np.asarray(bv, np.float64)
    ).astype(f)

    in_maps = []
    for c in range(NCORES):
        b, hg = c // 2, c % 2
        hs = hg * 512
        wvs = Wv[hs: hs + 512, :]  # [512 f, 1024 d]
        wv_t = np.ascontiguousarray(
            wvs.T.reshape(4, 2, 128, 2, 256).transpose(2, 0, 3, 1, 4)
        ).astype(b16)
        wos = np.ascontiguousarray(Wo.T[hs: hs + 512, :])  # [512 d, 1024 o]
        wo16 = np.ascontiguousarray(
            wos.reshape(4, 128, 8, 128).transpose(1, 0, 2, 3)
        ).astype(b16)
        vt = np.ascontiguousarray(np.asarray(value)[b].astype(f).T)
        xv_t = np.ascontiguousarray(
            vt.reshape(4, 2, 128, 16, 128).transpose(2, 3, 0, 1, 4)
        ).astype(b16)
        wb = np.zeros((128, 16), f)
        wb[:, 0:4] = bq[hs: hs + 512].reshape(4, 128).T
        wb[:, 4:8] = bk[hs: hs + 512].reshape(4, 128).T
        if hg == 0:
            wb[:, 8:16] = bo_eff.reshape(8, 128).T
        in_maps.append(
            {
                "xq8": x16(np.asarray(query)[b]),
                "xk8": x16(np.asarray(key)[b]),
                "xv": xv_t,
                "wq8": w16(Wq[hs: hs + 512, :]),
                "wk8": w16(Wk[hs: hs + 512, :]),
                "wv": wv_t,
                "wo8": wo16,
                "mask_t": mask_t,
                "wb": np.ascontiguousarray(wb),
                "ident": ident,
            }
        )
    return in_maps


def kernel(**inputs):
    from concourse.bass_utils import run_bass_kernel_spmd

    np_inputs = {k: np.asarray(v) for k, v in inputs.items()}
    in_maps = _prep(**np_inputs)
    nc = _get_nc()
    res = run_bass_kernel_spmd(nc, in_maps, list(range(NCORES)))
    out = np.empty((B, S, D), np.float32)
    for b in range(B):
        p0 = res.results[2 * b]["out_t"]
        p1 = res.results[2 * b + 1]["out_t"]
        out[b] = (p0 + p1).T
    return out


# revision 23
# speedup vs baseline: 1.2099x; 1.0288x over previous
"""Multi-head attention TRN2 kernel, 8-core SPMD, v4.

Sharding: core c owns batch b=c//2 and head-group hg=c%2 (8 of 16 heads).
Each core projects Q/K/V for its 8 heads over the full 2048-token sequence
of its batch, runs attention, and computes a PARTIAL output projection
(contraction over its 512 features). The host sums the two partial outputs
per batch -- no on-device collectives.

v4 redesign vs v3 (469 us -> target ~300 us):
- PV is computed in [queries, dims] orientation (probs tile as the
  stationary lhsT, V as moving rhs with a ones column for the softmax
  denominator): full 128 output partitions instead of 65, halving the
  tensor-engine cost of PV.
- Attention output is transposed back to [dims, queries] for the output
  projection with PE transposes; odd heads land on partitions 64..127
  directly via tile_position=(0, 64) (no SBUF-shift DMA).
- Normalization is a per-partition reciprocal + tensor_scalar multiply on
  the vector engine (no gpsimd partition_broadcast).
- Flash-style software pipeline: a short prefix (K/Q for head pair 0 and
  half the V chains), then per-(head, query-half) units where the scalar
  engine's exp (the pacing engine, ~1.04us per [128,1024] tile) is kept
  fed while the tensor engine interleaves scores, lagged PV, and
  background projection/output chains drained between score tiles.
- Output partials are written f32 (DMA has slack; removes bf16 rounding).
"""

import numpy as np
from collections import deque

B, S, D, H, DH = 4, 2048, 1024, 16, 64
NCORES = 8

V_PREFIX = 8   # V chains computed in the prefix (of 16)
LAG = 3        # PV lags scores by this many kc steps

_CACHE = {}


def _build():
    from contextlib import ExitStack

    import concourse.mybir as mybir
    import concourse.tile as tile
    from concourse import bacc

    f32 = mybir.dt.float32
    bf16 = mybir.dt.bfloat16
    EXP = mybir.ActivationFunctionType.Exp

    nc = bacc.Bacc(
        "TRN2",
        target_bir_lowering=False,
        debug=False,
        enable_asserts=False,
        num_devices=NCORES,
    )

    xq_d = nc.dram_tensor("xq8", [128, 4, 2, 2, 1024], bf16, kind="ExternalInput").ap()
    xk_d = nc.dram_tensor("xk8", [128, 4, 2, 2, 1024], bf16, kind="ExternalInput").ap()
    xv_d = nc.dram_tensor("xv", [128, 16, 4, 2, 128], bf16, kind="ExternalInput").ap()
    wq_d = nc.dram_tensor("wq8", [128, 4, 4, 2, 128], bf16, kind="ExternalInput").ap()
    wk_d = nc.dram_tensor("wk8", [128, 4, 4, 2, 128], bf16, kind="ExternalInput").ap()
    wv_d = nc.dram_tensor("wv", [128, 4, 2, 2, 256], bf16, kind="ExternalInput").ap()
    wo_d = nc.dram_tensor("wo8", [128, 4, 8, 128], bf16, kind="ExternalInput").ap()
    mk_d = nc.dram_tensor("mask_t", [128, 16, S], bf16, kind="ExternalInput").ap()
    wb_d = nc.dram_tensor("wb", [128, 16], f32, kind="ExternalInput").ap()
    id_d = nc.dram_tensor("ident", [128, 128], bf16, kind="ExternalInput").ap()
    out_d = nc.dram_tensor("out_t", [1024, S], f32, kind="ExternalOutput").ap()

    with tile.TileContext(nc) as tc:
        stk = ExitStack()

        konst = stk.enter_context(tc.tile_pool(name="konst", bufs=1))
        wb_sb = konst.tile([128, 16], f32, name="wb_sb")
        wq_sb = konst.tile([128, 4, 4, 2, 128], bf16, name="wq_sb")
        wk_sb = konst.tile([128, 4, 4, 2, 128], bf16, name="wk_sb")
        wv_sb = konst.tile([128, 4, 2, 2, 256], bf16, name="wv_sb")
        wo_sb = konst.tile([128, 4, 8, 128], bf16, name="wo_sb")
        id_sb = konst.tile([128, 128], bf16, name="id_sb")
        zz_sb = konst.tile([1, 512], bf16, name="zz_sb")

        mpool = stk.enter_context(tc.tile_pool(name="msk", bufs=1))
        mk = [mpool.tile([128, 1024], bf16, name=f"mk{kc}") for kc in range(16)]

        qkpool = stk.enter_context(tc.tile_pool(name="qk", bufs=1))
        qp = [qkpool.tile([128, 2, 1024], bf16, name=f"qp{fb}") for fb in range(4)]
        kp = [qkpool.tile([128, 16, 128], bf16, name=f"kp{fb}") for fb in range(4)]

        vpool = stk.enter_context(tc.tile_pool(name="vsb", bufs=1))
        v_sb = [vpool.tile([128, 8, 65], bf16, name=f"v{kc}") for kc in range(16)]

        xapool = stk.enter_context(tc.tile_pool(name="xatt", bufs=1))
        x_att = [xapool.tile([128, 4, 1024], bf16, name=f"xa{qh}") for qh in range(2)]

        xvpool = stk.enter_context(tc.tile_pool(name="xvs", bufs=6))
        pepool = stk.enter_context(tc.tile_pool(name="pe", bufs=16))
        xnpool = stk.enter_context(tc.tile_pool(name="xn", bufs=2))
        copool = stk.enter_context(tc.tile_pool(name="co", bufs=3))

        xbpool = stk.enter_context(tc.tile_pool(name="xb", bufs=3))

        stpool = stk.enter_context(tc.tile_pool(name="pst", bufs=2, space="PSUM"))
        pvpool = stk.enter_context(tc.tile_pool(name="ppv", bufs=2, space="PSUM"))
        trpool = stk.enter_context(tc.tile_pool(name="ptr", bufs=1, space="PSUM"))
        bgpool = stk.enter_context(tc.tile_pool(name="pbg", bufs=1, space="PSUM"))

        sp, gq, dv, pe_e, ac = nc.sync, nc.gpsimd, nc.vector, nc.tensor, nc.scalar

        # ---- constant / input DMAs ------------------------------------
        # The DMA engines behave as one serial FIFO; order everything by
        # when the data is first needed. sp/ac are HW-DGE (fast dispatch),
        # gq is SWDGE (~1us per dispatch).
        sp.dma_start(wk_sb[:, :, 0:2, :, :], wk_d[:, :, 0:2, :, :])
        ac.dma_start(wq_sb[:, :, 0:2, :, :], wq_d[:, :, 0:2, :, :])
        ac.dma_start(wb_sb[:], wb_d[:, :])
        gq.memset(zz_sb[:], 0.0)
        for kc in range(16):
            gq.memset(v_sb[kc][:, :, 64:65], 1.0)
        gq.dma_start(id_sb[:], id_d[:, :])

        # ---- helpers --------------------------------------------------
        chunks = {}

        def get_chunk(src_d, key, tb, deng):
            """Each x^T 512-token block is loaded exactly once (DMA engines
            are a serial resource in practice -- no re-loads)."""
            if key not in chunks:
                t = xbpool.tile([128, 4, 2, 512], bf16, tag="xb", name="xb")
                deng.dma_start(
                    t[:],
                    src_d[:, :, tb // 2, :,
                          (tb % 2) * 512:(tb % 2) * 512 + 512])
                chunks[key] = t
            return chunks[key]

        def q_wide(fb, th, ca, cb):
            """Q proj for all 1024 tokens of half th, head-pair fb (stpool)."""
            ps = stpool.tile([128, 1024], f32, tag="st", name="qw")
            for g, ch in enumerate((ca, cb)):
                for ici in range(8):
                    ic, i = ici // 2, ici % 2
                    pe_e.matmul(
                        ps[:, g * 512:(g + 1) * 512],
                        lhsT=wq_sb[:, ic, fb, i, :],
                        rhs=ch[:, ic, i, :],
                        start=(ici == 0), stop=(ici == 7),
                    )
            dv.tensor_scalar_add(
                qp[fb][:, th, :], ps[:], wb_sb[:, fb:fb + 1])

        def kpre(kb):
            """K proj chain for 512 keys of head pair 0 (stpool tile)."""
            ch = get_chunk(xk_d, ("kp", kb), kb, sp)
            ps = stpool.tile([128, 1024], f32, tag="st", name="kpre")
            for ici in range(8):
                ic, i = ici // 2, ici % 2
                pe_e.matmul(
                    ps[:, 0:512],
                    lhsT=wk_sb[:, ic, 0, i, :],
                    rhs=ch[:, ic, i, :],
                    start=(ici == 0), stop=(ici == 7),
                )
            dv.tensor_scalar_add(
                kp[0][:, kb * 4:(kb + 1) * 4, :],
                ps[:, 0:512].rearrange("p (a b) -> p a b", b=128),
                wb_sb[:, 4:5],
            )

        xv_t = [None] * 16

        def v_dma(kc, deng=None):
            xv_t[kc] = xvpool.tile([128, 4, 2, 128], bf16, tag="xv", name="xvt")
            (deng or gq).dma_start(xv_t[kc][:], xv_d[:, kc, :, :, :])

        # ---- background task generators (bgpool [128,512] chains) -----
        def k_group(kb):
            """K proj of key block kb for head pairs 1..3 (own chunk)."""
            ch = get_chunk(xk_d, ("k2", kb), kb, gq)
            yield
            for fb in range(1, 4):
                ps = bgpool.tile([128, 512], f32, tag="bg", name="kc_ps")
                for step in range(4):
                    for s in range(2):
                        ici = step * 2 + s
                        ic, i = ici // 2, ici % 2
                        pe_e.matmul(
                            ps[:],
                            lhsT=wk_sb[:, ic, fb, i, :],
                            rhs=ch[:, ic, i, :],
                            start=(ici == 0), stop=(ici == 7),
                        )
                    yield
                dv.tensor_scalar_add(
                    kp[fb][:, kb * 4:(kb + 1) * 4, :],
                    ps[:].rearrange("p (a b) -> p a b", b=128),
                    wb_sb[:, 4 + fb:5 + fb],
                )

        def q_group(tb, fbs, deng):
            """Q proj of token block tb for the given head pairs."""
            ch = get_chunk(xq_d, ("q", tb), tb, deng)
            yield
            th, hh = tb // 2, tb % 2
            for fb in fbs:
                ps = bgpool.tile([128, 512], f32, tag="bg", name="qc_ps")
                for step in range(4):
                    for s in range(2):
                        ici = step * 2 + s
                        ic, i = ici // 2, ici % 2
                        pe_e.matmul(
                            ps[:],
                            lhsT=wq_sb[:, ic, fb, i, :],
                            rhs=ch[:, ic, i, :],
                            start=(ici == 0), stop=(ici == 7),
                        )
                    yield
                dv.tensor_scalar_add(
                    qp[fb][:, th, hh * 512:(hh + 1) * 512], ps[:],
                    wb_sb[:, fb:fb + 1])

        def v_wide(kc0):
            ps = stpool.tile([128, 1024], f32, tag="st", name="vw")
            for g in range(2):
                kc = kc0 + g
                for ici in range(8):
                    ic, i = ici // 2, ici % 2
                    pe_e.matmul(
                        ps[:, g * 512:(g + 1) * 512],
                        lhsT=xv_t[kc][:, ic, i, :],
                        rhs=wv_sb[:, ic, :, i, :],
                        start=(ici == 0), stop=(ici == 7),
                    )
            for g in range(2):
                dv.tensor_copy(
                    v_sb[kc0 + g][:, :, 0:64],
                    ps[:, g * 512:(g + 1) * 512].rearrange(
                        "p (h f) -> p h f", f=64),
                )

        def v_chain(kc):
            if xv_t[kc] is None:
                v_dma(kc)
                yield
            ps = bgpool.tile([128, 512], f32, tag="bg", name="vc_ps")
            for step in range(4):
                for s in range(2):
                    ici = step * 2 + s
                    ic, i = ici // 2, ici % 2
                    pe_e.matmul(
                        ps[:],
                        lhsT=xv_t[kc][:, ic, i, :],
                        rhs=wv_sb[:, ic, :, i, :],
                        start=(ici == 0), stop=(ici == 7),
                    )
                yield
            dv.tensor_copy(
                v_sb[kc][:, :, 0:64],
                ps[:].rearrange("p (h f) -> p h f", f=64))

        def o_chain(qh, ofb, nh):
            ps = bgpool.tile([128, 512], f32, tag="bg", name="oc_ps")
            nsl = slice(nh * 512, (nh + 1) * 512)
            for ic in range(4):
                pe_e.matmul(
                    ps[:], lhsT=wo_sb[:, ic, ofb, :],
                    rhs=x_att[qh][:, ic, nsl],
                    start=(ic == 0), stop=(ic == 3),
                )
                if ic % 2 == 1:
                    yield
            co = copool.tile([128, 512], f32, tag="co", name="co")
            dv.tensor_scalar_add(co[:], ps[:], wb_sb[:, 8 + ofb:9 + ofb])
            (sp if nh == 0 else gq).dma_start(
                out_d[ofb * 128:(ofb + 1) * 128,
                      qh * 1024 + nh * 512: qh * 1024 + (nh + 1) * 512],
                co[:],
            )

        # ---- PE warm-up: the tensor engine only reaches full clock
        # after ~3us of continuous busy; burn junk matmuls into the (idle)
        # transpose bank while the first input DMAs are in flight.
        wu = trpool.tile([128, 512], f32, tag="tr", name="wu")
        for _ in range(16):
            pe_e.matmul(wu[:], lhsT=zz_sb[0:1, 0:128], rhs=zz_sb[0:1, 0:512],
                        start=True, stop=True)

        # ---- prefix: minimum before unit 0's first score tile ---------
        kpre(0)
        cq01 = [get_chunk(xq_d, ("q", 0), 0, sp),
                get_chunk(xq_d, ("q", 1), 1, ac)]
        ac.dma_start(wv_sb[:], wv_d[:, :, :, :, :])
        for kc in range(4):
            v_dma(kc, sp)
        for kc in range(4, 8):
            v_dma(kc, ac)
        for kc in range(8):
            ac.dma_start(mk[kc][:], mk_d[:, kc, 0:1024])
        q_wide(0, 0, cq01[0], cq01[1])
        v_wide(0)
        v_wide(2)
        for kc in range(8, 16):
            ac.dma_start(mk[kc][:], mk_d[:, kc, 0:1024])
        v_wide(4)
        v_wide(6)
        ac.dma_start(wk_sb[:, :, 2:4, :, :], wk_d[:, :, 2:4, :, :])
        ac.dma_start(wq_sb[:, :, 2:4, :, :], wq_d[:, :, 2:4, :, :])
        ac.dma_start(wo_sb[:], wo_d[:, :, :, :])

        inject0 = {1: lambda: kpre(1), 4: lambda: kpre(2),
                   8: lambda: kpre(3)}

        vgens = {kc: v_chain(kc) for kc in range(8, 16)}
        kg = {kb: k_group(kb) for kb in range(4)}
        qg0 = q_group(0, (1, 2, 3), sp)
        qg1 = q_group(1, (1, 2, 3), sp)
        qg2 = q_group(2, (0, 1, 2, 3), sp)
        qg3 = q_group(3, (0, 1, 2, 3), gq)
        bg = deque([qg0, qg1]
                   + [vgens[kc] for kc in range(8, 16)]
                   + [kg[0], kg[1], kg[2], kg[3], qg2, qg3])
        need_by_unit = {8: [qg2, qg3]}
        # scores of unit u (head pair fb>0) need kp[fb][:, kc]: force the
        # corresponding K group if the drains have not reached it yet
        kb_guard = kg

        def drain(n):
            done = 0
            while done < n and bg:
                try:
                    next(bg[0])
                    done += 1
                except StopIteration:
                    bg.popleft()

        def drain_gen(gen):
            """Force-finish one generator immediately."""
            for _ in gen:
                pass

        # ---- units ----------------------------------------------------
        # Cross-unit pipeline: unit u's PV tail, normalization, transposes
        # and x_att copy are emitted during unit u+1's first score steps so
        # the scalar engine is never starved at unit boundaries.
        pv_state = {}
        pe_q = deque()
        tr_cur = [None]

        def pv_step(u_, kc, pe):
            g_ = vgens.get(kc)
            if g_ is not None and g_.gi_frame is not None:
                drain_gen(g_)
            h_ = u_ % 8
            pv = pv_state[u_]
            for g in range(2):
                for qq in range(4):
                    pe_e.matmul(
                        pv[g][:, qq, :],
                        lhsT=pe[:, (4 * g + qq) * 128:(4 * g + qq + 1) * 128],
                        rhs=v_sb[kc][:, h_, :],
                        start=False, stop=False,
                        skip_group_check=True,
                    )

        def pv_ready(kc):
            g_ = vgens.get(kc)
            return g_ is None or g_.gi_frame is None

        def pv_flush(n=1, force=False):
            for _ in range(n):
                if pe_q and (force or pv_ready(pe_q[0][1])):
                    pv_step(*pe_q.popleft())

        def finish_unit(u_):
            qh, h = u_ // 8, u_ % 8
            fb, j = h // 2, h % 2
            pv = pv_state.pop(u_)
            if j == 0:
                tr_cur[0] = trpool.tile([128, 1024], bf16, tag="tr", name="tr")
            tr = tr_cur[0]
            for g in range(2):
                xn = xnpool.tile([128, 4, 64], bf16, tag=f"xn{g}", name="xn")
                rc = xnpool.tile([128, 4, 1], f32, tag=f"rc{g}", name="rc")
                dv.reciprocal(rc[:], pv[g][:, :, 64:65])
                for qq in range(4):
                    dv.tensor_scalar_mul(
                        xn[:, qq, :], pv[g][:, qq, 0:64], rc[:, qq, :])
                for qq in range(4):
                    c = 4 * g + qq
                    pe_e.transpose(
                        tr[64 * j:64 * j + 64, c * 128:(c + 1) * 128],
                        xn[:, qq, :], id_sb[:],
                        tile_position=(0, 64 * j),
                    )
            if j == 1:
                dv.tensor_copy(x_att[qh][:, fb, :], tr[:])

        for u in range(16):
            qh, h = u // 8, u % 8
            fb, j = h // 2, h % 2
            for g_ in need_by_unit.pop(u, ()):
                # qp writes must be emitted before their readers
                drain_gen(g_)
            if u == 2:
                drain_gen(qg0)
                drain_gen(qg1)
            pv_state[u] = [
                pvpool.tile([128, 4, 65], f32, tag="pv", name=f"pv{g}")
                for g in range(2)]
            for g in range(2):
                # claim the PSUM bank: start=True marks the whole 2KB zero
                # region pending and this instruction writes zeros over the
                # accumulator bytes; the PV matmuls then accumulate with
                # start=False (one hardware group per bank).
                pe_e.matmul(
                    pv_state[u][g][:],
                    lhsT=zz_sb[0:1, 0:128], rhs=zz_sb[0:1, 0:260],
                    start=True, stop=True,
                )
            budget = 2 if u <= 3 else 1
            for kc in range(16):
                # step order: background atoms and PV first, score matmuls
                # last -- the score's wait on the st-pool rotation (exp of
                # kc-2) is then hidden behind the step's other PE work.
                if u == 0 and kc in inject0:
                    inject0[kc]()
                drain(budget)
                if len(pe_q) > LAG:
                    pv_flush(2 if len(pe_q) > LAG + 2 else 1)
                while len(pv_state) > 1:
                    u0_ = min(pv_state)
                    if u0_ < u and (not pe_q or pe_q[0][0] > u0_):
                        finish_unit(u0_)
                    else:
                        break
                if fb > 0 and u in (2, 3):
                    g_ = kb_guard.get(kc // 4)
                    if g_ is not None and g_.gi_frame is not None:
                        drain_gen(g_)
                st = stpool.tile([128, 1024], f32, tag="st", name="st")
                for nhs in range(2):
                    nsl = slice(nhs * 512, (nhs + 1) * 512)
                    pe_e.matmul(
                        st[:, nsl],
                        lhsT=kp[fb][64 * j:64 * j + 64, kc, :],
                        rhs=qp[fb][64 * j:64 * j + 64, qh, nsl],
                        start=True, stop=True,
                    )
                pe = pepool.tile([128, 1024], bf16, tag="pe", name="pe")
                ac.activation(pe[:], st[:], EXP, scale=0.125)
                dv.tensor_mul(pe[:], pe[:], mk[kc][:])
                if qh == 0 and h == 7:
                    sp.dma_start(mk[kc][:], mk_d[:, kc, 1024:2048])
                pe_q.append((u, kc, pe))
                if u == 8 and kc == LAG + 2:
                    for ofb in range(8):
                        for nh in range(2):
                            bg.append(o_chain(0, ofb, nh))
        while pe_q:
            pv_flush(1, force=True)
            drain(1)
        finish_unit(15)
        while bg:
            drain_gen(bg.popleft())

        # ---- tail: qh1 output projection, 4-deep psum rotation --------
        tail_slots = [(stpool, "st"), (bgpool, "bg"), (stpool, "st"),
                      (trpool, "tr")]
        tail_q = [sp, ac, gq, sp]
        for i in range(16):
            ofb, nh = i // 2, i % 2
            pool_, tag_ = tail_slots[i % 4]
            ps = pool_.tile([128, 512], f32, tag=tag_, name="ow")
            for ic in range(4):
                pe_e.matmul(
                    ps[:], lhsT=wo_sb[:, ic, ofb, :],
                    rhs=x_att[1][:, ic, nh * 512:(nh + 1) * 512],
                    start=(ic == 0), stop=(ic == 3),
                )
            co = copool.tile([128, 512], f32, tag="co", name="co")
            dv.tensor_scalar_add(co[:], ps[:], wb_sb[:, 8 + ofb:9 + ofb])
            tail_q[i % 4].dma_start(
                out_d[ofb * 128:(ofb + 1) * 128,
                      1024 + nh * 512:1024 + (nh + 1) * 512],
                co[:],
            )
        stk.close()

    nc.compile()
    return nc


def _get_nc():
    if "nc" not in _CACHE:
        _CACHE["nc"] = _build()
    return _CACHE["nc"]


def _prep(query, key, value, mask, Wq, bq, Wk, bk, Wv, bv, Wo, bo):
    import ml_dtypes

    f = np.float32
    b16 = ml_dtypes.bfloat16

    def x16(x2d):  # [2048 t, 1024 d] -> [128 p, 4 ic, 2 th, 2 i, 1024 t]
        xt = np.ascontiguousarray(np.asarray(x2d, f).T)  # [1024 d, 2048]
        a = xt.reshape(4, 2, 128, 2, 1024)  # ic, i, p, th, t
        return np.ascontiguousarray(a.transpose(2, 0, 3, 1, 4)).astype(b16)

    def w16(Ws):  # [512 f, 1024 d] -> [128 p, 4 ic, 4 fb, 2 i, 128 f]
        wt = np.ascontiguousarray(np.asarray(Ws, f).T)  # [1024 d, 512 f]
        return np.ascontiguousarray(
            wt.reshape(4, 2, 128, 4, 128).transpose(2, 0, 3, 1, 4)
        ).astype(b16)

    m2 = np.asarray(mask)[0, 0]  # [Sq, Sk]
    mask_t = np.ascontiguousarray(
        np.ascontiguousarray(m2.T).reshape(16, 128, S).transpose(1, 0, 2)
    ).astype(b16)
    ident = np.eye(128).astype(b16)

    Wq, Wk, Wv, Wo = (np.asarray(a, f) for a in (Wq, Wk, Wv, Wo))
    bq, bk, bv, bo = (np.asarray(a, f) for a in (bq, bk, bv, bo))
    bo_eff = (
        np.asarray(bo, np.float64)
        + np.asarray(Wo, np.float64) @ np.asarray(bv, np.float64)
    ).astype(f)

    in_maps = []
    for c in range(NCORES):
        b, hg = c // 2, c % 2
        hs = hg * 512
        wvs = Wv[hs: hs + 512, :]  # [512 f, 1024 d]
        wv_t = np.ascontiguousarray(
            wvs.T.reshape(4, 2, 128, 2, 256).transpose(2, 0, 3, 1, 4)
        ).astype(b16)
        wos = np.ascontiguousarray(Wo.T[hs: hs + 512, :])  # [512 d, 1024 o]
        wo16 = np.ascontiguousarray(
            wos.reshape(4, 128, 8, 128).transpose(1, 0, 2, 3)
        ).astype(b16)
        vt = np.ascontiguousarray(np.asarray(value)[b].astype(f).T)
        xv_t = np.ascontiguousarray(
            vt.reshape(4, 2, 128, 16, 128).transpose(2, 3, 0, 1, 4)
        ).astype(b16)
        wb = np.zeros((128, 16), f)
        wb[:, 0:4] = bq[hs: hs + 512].reshape(4, 128).T
        wb[:, 4:8] = bk[hs: hs + 512].reshape(4, 128).T
        if hg == 0:
            wb[:, 8:16] = bo_eff.reshape(8, 128).T
        in_maps.append(
            {
                "xq8": x16(np.asarray(query)[b]),
                "xk8": x16(np.asarray(key)[b]),
                "xv": xv_t,
                "wq8": w16(Wq[hs: hs + 512, :]),
                "wk8": w16(Wk[hs: hs + 512, :]),
                "wv": wv_t,
                "wo8": wo16,
                "mask_t": mask_t,
                "wb": np.ascontiguousarray(wb),
                "ident": ident,
            }
        )
    return in_maps


def kernel(**inputs):
    from concourse.bass_utils import run_bass_kernel_spmd

    np_inputs = {k: np.asarray(v) for k, v in inputs.items()}
    in_maps = _prep(**np_inputs)
    nc = _get_nc()
    res = run_bass_kernel_spmd(nc, in_maps, list(range(NCORES)))
    out = np.empty((B, S, D), np.float32)
    for b in range(B):
        p0 = res.results[2 * b]["out_t"]
        p1 = res.results[2 * b + 1]["out_t"]
        out[b] = (p0 + p1).T
    return out
